# revision 24
# baseline (speedup 1.0000x reference)
"""GQA kernel for Trainium2 (Bass/Tile), 8 NeuronCores — v4.

Sharding: core c -> batch b=c//4, kv-head pair j=c%4 (kv heads 2j,2j+1,
q heads 8j..8j+7).  Each core computes out[b, :, 512j:512(j+1)].

Structure (all compute bf16 / fp32-accum; fp8 was tested and rejected —
QK-fp8 alone costs 3.8e-2 rel err, P-fp8 5.5e-2, both over budget):
  * q-chunk-major attention: for j (512-wide q chunk) -> for pair
    (head m & m+4) -> for ki (k-tile): S^T chunk-pair, exp, PV.
  * S^T chunk-pair via row-tiled CONCURRENT matmuls: head m uses
    kt/qt partitions 0:64 (tile_position (0,0)), head m+4 uses 64:128
    ((64,0)) -> full PE array despite K=64 contraction.
  * causal mask: DVE adds -3e7 onto the diagonal 128x128 S^T block in
    PSUM before exp (exp then gives exact 0).
  * one exp ACTIVATE per item covers both heads ([128,2,w] strided AP
    over the 2-bank S pair); output bf16 into quad-plane pt tiles.
  * PV with V' stationary (V k-tile 64 cols + ones column, M=65),
    P^T chunk moving (N<=512) -> O^T[d|den, q] accumulates in PSUM
    over ki.  Stream-bound instead of LDWEIGHTS-bound.
  * V computed as V^T (wv stationary, x moving; stream-efficient),
    then PE-transposed per 128-col block into V' k-tiles.
  * output stays in O^T layout [8 heads x 65, T]; host divides by the
    denominator row and transposes.
  * projections split into 512-col chunk groups, woven through the
    attention stream as deadline-ordered fillers; input DMA ordered
    wk -> x.tc0 -> wq -> x rest -> wv across both HWDGE queues.
"""

import sys

for _p in ("/opt/trn_rl_repo",):
    if _p not in sys.path:
        sys.path.insert(0, _p)

import numpy as np
import ml_dtypes

import concourse.bass as bass
import concourse.tile as tile
from concourse import bacc, mybir
from concourse.bass_utils import run_bass_kernel_spmd
from concourse.masks import make_lower_triangular, make_identity

BF16 = mybir.dt.bfloat16
F32 = mybir.dt.float32
AF = mybir.ActivationFunctionType
ALU = mybir.AluOpType

D = 2048
HS = 64
SCALE = 0.125       # 1/sqrt(HS)
LAG = 4             # S->PV software-pipeline lag (items)


def _emit_body(tc, aps, T):
    nc = tc.nc
    NT = T // 128            # k tiles
    NJ = T // 512            # q chunks
    ND = D // 128            # contraction chunks

    xT, wqT, wkT, wvT, cosr, sins, out = aps

    import contextlib
    ctx = tc._kernel_exitstack = contextlib.ExitStack()

    pers = ctx.enter_context(tc.tile_pool(name="pers", bufs=1))
    rp = ctx.enter_context(tc.tile_pool(name="rope", bufs=2))
    ptp = ctx.enter_context(tc.tile_pool(name="ptp", bufs=4))
    evp = ctx.enter_context(tc.tile_pool(name="evp", bufs=2))
    sp = ctx.enter_context(tc.tile_pool(name="spsum", bufs=2, space="PSUM"))
    po = ctx.enter_context(tc.tile_pool(name="opsum", bufs=1, space="PSUM"))
    aux = ctx.enter_context(tc.tile_pool(name="aux", bufs=1, space="PSUM"))

    # ---- input DMA: both HWDGE queues; priority order so K/Q0 start early
    wqTs, wkTs, wvTs = [], [], []
    xTs = [[None] * NJ for _ in range(ND)]
    for di in range(ND):
        t = pers.tile([128, 128], BF16, tag=f"wk{di}", name=f"wk{di}")
        nc.sync.dma_start(out=t[:], in_=wkT[di * 128:(di + 1) * 128, :])
        wkTs.append(t)
    for tcI in range(NJ):
        for di in range(ND):
            xTs[di][tcI] = pers.tile([128, 512], BF16, tag=f"x{di}_{tcI}",
                                     name=f"x{di}_{tcI}")

    def dma_x(q, di, tcI):
        eng = nc.sync if q == 0 else nc.gpsimd
        eng.dma_start(out=xTs[di][tcI][:],
                      in_=xT[di * 128:(di + 1) * 128,
                             tcI * 512:(tcI + 1) * 512])

    for di in range(ND):
        wqTs.append(pers.tile([128, 512], BF16, tag=f"wq{di}",
                              name=f"wq{di}"))
        wvTs.append(pers.tile([128, 128], BF16, tag=f"wv{di}",
                              name=f"wv{di}"))
    # sync (HWDGE): cos/sin chunk0 -> x.tc0 -> wq m0 -> wv -> xe.tc1 ->
    #               wq m1-3 -> xe.tc2-3.
    # gpsimd (SWDGE, slower, late-needed only): cos/sin rest, xo.tc1-3.
    cosr_t = pers.tile([128, T], BF16, tag="cosr", name="cosr")
    sins_t = pers.tile([128, T], BF16, tag="sins", name="sins")
    nc.sync.dma_start(out=cosr_t[:, 0:512], in_=cosr[:, 0:512])
    nc.sync.dma_start(out=sins_t[:, 0:512], in_=sins[:, 0:512])
    nc.gpsimd.dma_start(out=cosr_t[:, 512:T], in_=cosr[:, 512:T])
    nc.gpsimd.dma_start(out=sins_t[:, 512:T], in_=sins[:, 512:T])
    for di in range(ND):
        dma_x(0, di, 0)
    for di in range(ND):
        nc.sync.dma_start(out=wqTs[di][:, 0:128],
                          in_=wqT[di * 128:(di + 1) * 128, 0:128])
    for di in range(ND):
        nc.sync.dma_start(out=wvTs[di][:],
                          in_=wvT[di * 128:(di + 1) * 128, :])
    for di in range(0, ND, 2):
        dma_x(0, di, 1)
    for di in range(1, ND, 2):
        dma_x(1, di, 1)
    for di in range(ND):
        nc.sync.dma_start(out=wqTs[di][:, 128:512],
                          in_=wqT[di * 128:(di + 1) * 128, 128:512])
    for tcI in range(2, NJ):
        for di in range(0, ND, 2):
            dma_x(0, di, tcI)
        for di in range(1, ND, 2):
            dma_x(1, di, tcI)

    madd = pers.tile([128, 128], F32, tag="madd", name="madd")
    make_lower_triangular(nc, madd[:], val=-3.0e7, diag=False)
    idty = pers.tile([128, 128], BF16, tag="idty", name="idty")
    make_identity(nc, idty[:])

    qts = [pers.tile([128, T], BF16, tag=f"qt{m}", name=f"qt{m}")
           for m in range(4)]
    kt = pers.tile([128, T], BF16, tag="kt", name="kt")
    vxT = pers.tile([128, T], BF16, tag="vxT", name="vxT")
    vts = []
    for ti in range(NT):
        v = pers.tile([128, 130], BF16, tag=f"v{ti}", name=f"v{ti}")
        nc.vector.memset(v[:, 64:65], 1.0)
        nc.vector.memset(v[:, 129:130], 1.0)
        vts.append(v)

    # ---- helpers ----
    def rope_chunk(tgt, tcI, mirror=None):
        sl = slice(tcI * 512, (tcI + 1) * 512)
        swp = rp.tile([128, 512], BF16, tag="swp", name="swp")
        for (a, b) in ((0, 32), (32, 0), (64, 96), (96, 64)):
            nc.sync.dma_start(out=swp[a:a + 32, :], in_=tgt[b:b + 32, sl])
        tmp = rp.tile([128, 512], BF16, tag="tmp", name="tmp")
        nc.vector.tensor_tensor(out=tmp[:], in0=tgt[:, sl], in1=cosr_t[:, sl],
                                op=ALU.mult)
        nc.vector.tensor_tensor(out=swp[:], in0=swp[:], in1=sins_t[:, sl],
                                op=ALU.mult)
        nc.vector.tensor_tensor(out=tgt[:, sl], in0=tmp[:], in1=swp[:],
                                op=ALU.add)
        del mirror

    def make_quanta(kind, m, tcI):
        """One projection group (16 di-MMs into one [128,512] psum +
        evac), split into 4 quanta of 4 MMs."""
        state = {}

        def quantum(k):
            if k == 0:
                state["ps"] = aux.tile([128, 512], F32, tag="pj", name="pj")
            ps = state["ps"]
            for di in range(4 * k, 4 * k + 4):
                if kind == "q":
                    w = wqTs[di][:, m * 128:(m + 1) * 128]
                elif kind == "k":
                    w = wkTs[di][:]
                else:
                    w = wvTs[di][:]
                nc.tensor.matmul(ps[:], w, xTs[di][tcI][:],
                                 start=(di == 0), stop=(di == ND - 1))
            if k == 3:
                dst = {"q": qts[m] if m is not None else None,
                       "k": kt, "v": vxT}[kind]
                nc.vector.tensor_copy(dst[:, tcI * 512:(tcI + 1) * 512], ps[:])

        return [lambda kk=k: quantum(kk) for k in range(4)]

    def v_tr(ti):
        """PE-transpose one 128-col block of V^T into V' k-tile ti."""
        tp = aux.tile([128, 512], BF16, tag="vtr", name="vtr")
        nc.tensor.transpose(tp[:, 0:128], vxT[:, ti * 128:(ti + 1) * 128],
                            idty[:])
        nc.vector.tensor_copy(vts[ti][:, 0:64], tp[:, 0:64])
        nc.vector.tensor_copy(vts[ti][:, 65:129], tp[:, 64:128])

    # ---- prologue ----
    for f in make_quanta("k", None, 0):
        f()
    rope_chunk(kt, 0)
    for f in make_quanta("q", 0, 0):
        f()
    rope_chunk(qts[0], 0)
    for f in make_quanta("v", None, 0):
        f()
    for ti in range(4):
        v_tr(ti)

    # ---- filler queue ----
    def g_of(j, p):
        return sum(4 * jj + 4 for jj in range(j)) * 4 + (4 * j + 4) * p

    fillers = []

    def add_group(d, kind, m, tcI, rope_tgt=None, mirror=None):
        for f in make_quanta(kind, m, tcI):
            fillers.append((d, f))
        if rope_tgt is not None:
            fillers.append((d, lambda t=rope_tgt, c=tcI, mi=mirror:
                            rope_chunk(t, c, mi)))

    for tcI in range(1, NJ):
        d0 = g_of(tcI, 0)
        add_group(d0 - 8, "k", None, tcI, kt)
        add_group(d0 - 4, "q", 0, tcI, qts[0])
        add_group(d0 + 2, "v", None, tcI)
        for ti in range(4 * tcI, 4 * tcI + 4):
            fillers.append((d0 + 3, lambda t=ti: v_tr(t)))
    for m in range(1, 4):
        for tcI in range(NJ):
            add_group(g_of(tcI, m) - 2, "q", m, tcI, qts[m])
    fillers.sort(key=lambda df: df[0])
    fq = list(fillers)

    # ---- attention stream ----
    items = [(j, p, ki) for j in range(NJ) for p in range(4)
             for ki in range(4 * j + 4)]
    pend = []
    acc = {}
    cur_pt = [None]

    def emit_S(j, p, ki):
        o = max(0, 128 * ki - 512 * j)
        s = sp.tile([128, 1024], F32, tag="s", name="s")
        q0 = 512 * j + o
        q1 = 512 * (j + 1)
        kc = slice(ki * 128, (ki + 1) * 128)
        nc.tensor.matmul(s[:, o:512], kt[0:64, kc],
                         qts[p][0:64, q0:q1], start=True, stop=True)
        nc.tensor.matmul(s[:, 512 + o:1024], kt[64:128, kc],
                         qts[p][64:128, q0:q1], start=True, stop=True)
        if ki >= 4 * j:  # diagonal k-tile: add -3e7 on the q<k triangle
            nc.vector.tensor_tensor(out=s[:, o:o + 128], in0=s[:, o:o + 128],
                                    in1=madd[:], op=ALU.add)
            nc.vector.tensor_tensor(out=s[:, 512 + o:512 + o + 128],
                                    in0=s[:, 512 + o:512 + o + 128],
                                    in1=madd[:], op=ALU.add)
        # pt pair tile: [128, 2048] bf16 = [A_e|A_o|B_e|B_o] planes
        if ki % 2 == 0:
            cur_pt[0] = ptp.tile([128, 2048], BF16, tag="pt", name="pt")
        pt = cur_pt[0]
        src = s[:].rearrange("p (g c) -> p g c", g=2)[:, :, o:512]
        half = pt[:].rearrange("p (g c) -> p g c", g=2)  # halves A / B
        if ki % 2 == 0:
            dst = half[:, :, o:512]
        else:
            dst = half[:, :, 512 + o:1024]
        nc.scalar.activation(dst, src, AF.Exp, scale=SCALE)
        return pt

    def emit_PV(j, p, ki, pt):
        o = max(0, 128 * ki - 512 * j)
        if ki == 0:
            acc[(j, p)] = (po.tile([65, 512], F32, tag="oA", name="oA"),
                           po.tile([65, 512], F32, tag="oB", name="oB"))
        oA, oB = acc[(j, p)]
        last = (ki == 4 * j + 3)
        quads = pt[:].rearrange("p (g c) -> p g c", g=4)
        qa = ki % 2          # plane within pt for this ki
        nc.tensor.matmul(oA[:, o:512], vts[ki][:, 0:65],
                         quads[:, qa, o:512],
                         start=(ki == 0), stop=last)
        nc.tensor.matmul(oB[:, o:512], vts[ki][:, 65:130],
                         quads[:, 2 + qa, o:512],
                         start=(ki == 0), stop=last)
        if last:
            del acc[(j, p)]
            otA = evp.tile([65, 512], F32, tag="oa", name="oa")
            nc.vector.tensor_copy(otA[:], oA[:])
            nc.sync.dma_start(
                out=out[65 * p:65 * (p + 1), 512 * j:512 * (j + 1)],
                in_=otA[:])
            otB = evp.tile([65, 512], F32, tag="ob", name="ob")
            nc.vector.tensor_copy(otB[:], oB[:])
            nc.sync.dma_start(
                out=out[65 * (p + 4):65 * (p + 5), 512 * j:512 * (j + 1)],
                in_=otB[:])

    for g, (j, p, ki) in enumerate(items):
        pt = emit_S(j, p, ki)
        pend.append((j, p, ki, pt))
        if len(pend) > LAG:
            emit_PV(*pend.pop(0))
        emitted = 0
        while fq and (emitted < 1 or fq[0][0] <= g + 2):
            fq.pop(0)[1]()
            emitted += 1
    while pend:
        emit_PV(*pend.pop(0))
    while fq:
        fq.pop(0)[1]()

    ctx.close()


def build_program(T=2048, num_devices=8):
    nc = bacc.Bacc("TRN2", target_bir_lowering=False, debug=False,
                   num_devices=num_devices)
    xT = nc.dram_tensor("xT", (D, T), BF16, kind="ExternalInput").ap()
    wqT = nc.dram_tensor("wqT", (D, 512), BF16, kind="ExternalInput").ap()
    wkT = nc.dram_tensor("wkT", (D, 128), BF16, kind="ExternalInput").ap()
    wvT = nc.dram_tensor("wvT", (D, 128), BF16, kind="ExternalInput").ap()
    cosr = nc.dram_tensor("cosr", (128, T), BF16, kind="ExternalInput").ap()
    sins = nc.dram_tensor("sins", (128, T), BF16, kind="ExternalInput").ap()
    out = nc.dram_tensor("out", (520, T), F32, kind="ExternalOutput").ap()
    with tile.TileContext(nc) as tc:
        _emit_body(tc, (xT, wqT, wkT, wvT, cosr, sins, out), T)
    nc.compile()
    return nc


# ---------------- host side ----------------

def _qperm(j):
    rows = []
    for m in range(4):
        for r in range(128):
            h = m if r < 64 else m + 4
            d = 2 * (r % 32) + (1 if (r % 64) >= 32 else 0)
            rows.append((8 * j + h) * 64 + d)
    return np.array(rows)


def _kperm(j):
    rows = []
    for kv in range(2):
        for r in range(64):
            d = 2 * (r % 32) + (1 if r >= 32 else 0)
            rows.append((2 * j + kv) * 64 + d)
    return np.array(rows)


def make_core_inputs(x, Wq, Wk, Wv, cos, sin):
    """Per-core input dicts (host prep). x: [B,T,D]."""
    bf = ml_dtypes.bfloat16
    B, T, _ = x.shape
    xTb = [np.ascontiguousarray(x[b].T).astype(bf) for b in range(B)]
    cosT = np.ascontiguousarray(cos.T.astype(np.float32))
    sinT = np.ascontiguousarray(sin.T.astype(np.float32))
    cosr = np.tile(cosT, (4, 1)).astype(bf)
    sgn = np.repeat(np.array([-1.0, 1.0, -1.0, 1.0], np.float32), 32)
    sins = (np.tile(sinT, (4, 1)) * sgn[:, None]).astype(bf)
    maps = []
    for c in range(8):
        b, j = c // 4, c % 4
        maps.append({
            "xT": xTb[b],
            "wqT": np.ascontiguousarray(Wq[_qperm(j)].T).astype(bf),
            "wkT": np.ascontiguousarray(Wk[_kperm(j)].T).astype(bf),
            "wvT": np.ascontiguousarray(Wv[128 * j:128 * (j + 1)].T).astype(bf),
            "cosr": cosr,
            "sins": sins,
        })
    return maps


def unshard(results, B=2, T=2048):
    """Combine per-core O^T blocks into the full [B, T, 2048] output."""
    out = np.empty((B, T, 2048), np.float32)
    for c in range(8):
        b, j = c // 4, c % 4
        blk = results[c]["out"].reshape(8, 65, T)
        O = blk[:, :64, :] / blk[:, 64:65, :]
        out[b, :, 512 * j:512 * (j + 1)] = (
            O.transpose(2, 0, 1).reshape(T, 512))
    return out


_CACHE = {}


def _get_program():
    if "nc" not in _CACHE:
        _CACHE["nc"] = build_program(T=2048, num_devices=8)
    return _CACHE["nc"]


def run_on_hw(in_maps, trace=False, tmpdir=None):
    nc = _get_program()
    return run_bass_kernel_spmd(nc, in_maps, list(range(8)), trace=trace,
                                tmpdir=tmpdir)


def kernel(x, Wq, Wk, Wv, cos, sin):
    x = np.asarray(x, np.float32)
    Wq = np.asarray(Wq, np.float32)
    Wk = np.asarray(Wk, np.float32)
    Wv = np.asarray(Wv, np.float32)
    cos = np.asarray(cos, np.float32)
    sin = np.asarray(sin, np.float32)
    maps = make_core_inputs(x, Wq, Wk, Wv, cos, sin)
    res = run_on_hw(maps, trace=False)
    return unshard(res.results, x.shape[0], x.shape[1])


# revision 30
# speedup vs baseline: 1.1622x; 1.1622x over previous
"""GQA kernel for Trainium2 (Bass/Tile), 8 NeuronCores — v4.

Sharding: core c -> batch b=c//4, kv-head pair j=c%4 (kv heads 2j,2j+1,
q heads 8j..8j+7).  Each core computes out[b, :, 512j:512(j+1)].

Structure (all compute bf16 / fp32-accum; fp8 was tested and rejected —
QK-fp8 alone costs 3.8e-2 rel err, P-fp8 5.5e-2, both over budget):
  * q-chunk-major attention: for j (512-wide q chunk) -> for pair
    (head m & m+4) -> for ki (k-tile): S^T chunk-pair, exp, PV.
  * S^T chunk-pair via row-tiled CONCURRENT matmuls: head m uses
    kt/qt partitions 0:64 (tile_position (0,0)), head m+4 uses 64:128
    ((64,0)) -> full PE array despite K=64 contraction.
  * causal mask: DVE adds -3e7 onto the diagonal 128x128 S^T block in
    PSUM before exp (exp then gives exact 0).
  * one exp ACTIVATE per item covers both heads ([128,2,w] strided AP
    over the 2-bank S pair); output bf16 into quad-plane pt tiles.
  * PV with V' stationary (V k-tile 64 cols + ones column, M=65),
    P^T chunk moving (N<=512) -> O^T[d|den, q] accumulates in PSUM
    over ki.  Stream-bound instead of LDWEIGHTS-bound.
  * V computed as V^T (wv stationary, x moving; stream-efficient),
    then PE-transposed per 128-col block into V' k-tiles.
  * output stays in O^T layout [8 heads x 65, T]; host divides by the
    denominator row and transposes.
  * projections split into 512-col chunk groups, woven through the
    attention stream as deadline-ordered fillers; input DMA ordered
    wk -> x.tc0 -> wq -> x rest -> wv across both HWDGE queues.
"""

import sys

for _p in ("/opt/trn_rl_repo",):
    if _p not in sys.path:
        sys.path.insert(0, _p)

import numpy as np
import ml_dtypes

import concourse.bass as bass
import concourse.tile as tile
from concourse import bacc, mybir
from concourse.bass_utils import run_bass_kernel_spmd
from concourse.masks import make_lower_triangular, make_identity

BF16 = mybir.dt.bfloat16
F32 = mybir.dt.float32
AF = mybir.ActivationFunctionType
ALU = mybir.AluOpType

D = 2048
HS = 64
SCALE = 0.125       # 1/sqrt(HS)
LAG = 4             # S->PV software-pipeline lag (items)


def _emit_body(tc, aps, T):
    nc = tc.nc
    NT = T // 128            # k tiles
    NJ = T // 512            # q chunks
    ND = D // 128            # contraction chunks

    xT, wqT, wkT, wvT, cosr, sins, p32, out = aps

    import contextlib
    ctx = tc._kernel_exitstack = contextlib.ExitStack()

    pers = ctx.enter_context(tc.tile_pool(name="pers", bufs=1))
    rp = ctx.enter_context(tc.tile_pool(name="rope", bufs=2))
    ptp = ctx.enter_context(tc.tile_pool(name="ptp", bufs=4))
    evp = ctx.enter_context(tc.tile_pool(name="evp", bufs=2))
    sp = ctx.enter_context(tc.tile_pool(name="spsum", bufs=2, space="PSUM"))
    po = ctx.enter_context(tc.tile_pool(name="opsum", bufs=1, space="PSUM"))
    aux = ctx.enter_context(tc.tile_pool(name="aux", bufs=1, space="PSUM"))

    # ---- input DMA: both HWDGE queues; priority order so K/Q0 start early
    wqTs, wkTs, wvTs = [], [], []
    xTs = [[None] * NJ for _ in range(ND)]
    for di in range(ND):
        t = pers.tile([128, 128], BF16, tag=f"wk{di}", name=f"wk{di}")
        nc.sync.dma_start(out=t[:], in_=wkT[di * 128:(di + 1) * 128, :])
        wkTs.append(t)
    for tcI in range(NJ):
        for di in range(ND):
            xTs[di][tcI] = pers.tile([128, 512], BF16, tag=f"x{di}_{tcI}",
                                     name=f"x{di}_{tcI}")

    def dma_x(q, di, tcI):
        eng = nc.sync if q == 0 else nc.gpsimd
        eng.dma_start(out=xTs[di][tcI][:],
                      in_=xT[di * 128:(di + 1) * 128,
                             tcI * 512:(tcI + 1) * 512])

    for di in range(ND):
        wqTs.append(pers.tile([128, 512], BF16, tag=f"wq{di}",
                              name=f"wq{di}"))
        wvTs.append(pers.tile([128, 128], BF16, tag=f"wv{di}",
                              name=f"wv{di}"))
    # sync (HWDGE): cos/sin chunk0 -> x.tc0 -> wq m0 -> wv -> xe.tc1 ->
    #               wq m1-3 -> xe.tc2-3.
    # gpsimd (SWDGE, slower, late-needed only): cos/sin rest, xo.tc1-3.
    cosr_t = pers.tile([128, T], BF16, tag="cosr", name="cosr")
    sins_t = pers.tile([128, T], BF16, tag="sins", name="sins")
    p32t = pers.tile([128, 128], BF16, tag="p32", name="p32")
    nc.sync.dma_start(out=p32t[:], in_=p32[:, :])
    nc.sync.dma_start(out=cosr_t[:, 0:512], in_=cosr[:, 0:512])
    nc.sync.dma_start(out=sins_t[:, 0:512], in_=sins[:, 0:512])
    nc.gpsimd.dma_start(out=cosr_t[:, 512:T], in_=cosr[:, 512:T])
    nc.gpsimd.dma_start(out=sins_t[:, 512:T], in_=sins[:, 512:T])
    for di in range(ND):
        dma_x(0, di, 0)
    for di in range(ND):
        nc.sync.dma_start(out=wqTs[di][:, 0:128],
                          in_=wqT[di * 128:(di + 1) * 128, 0:128])
    for di in range(ND):
        nc.sync.dma_start(out=wvTs[di][:],
                          in_=wvT[di * 128:(di + 1) * 128, :])
    for di in range(0, ND, 2):
        dma_x(0, di, 1)
    for di in range(1, ND, 2):
        dma_x(1, di, 1)
    for di in range(ND):
        nc.sync.dma_start(out=wqTs[di][:, 128:512],
                          in_=wqT[di * 128:(di + 1) * 128, 128:512])
    for tcI in range(2, NJ):
        for di in range(0, ND, 2):
            dma_x(0, di, tcI)
        for di in range(1, ND, 2):
            dma_x(1, di, tcI)

    madd = pers.tile([128, 128], F32, tag="madd", name="madd")
    make_lower_triangular(nc, madd[:], val=-3.0e7, diag=False)
    idty = pers.tile([128, 128], BF16, tag="idty", name="idty")
    make_identity(nc, idty[:])

    qts = [pers.tile([128, T], BF16, tag=f"qt{m}", name=f"qt{m}")
           for m in range(4)]
    kt = pers.tile([128, T], BF16, tag="kt", name="kt")
    vxT = pers.tile([128, T], BF16, tag="vxT", name="vxT")
    vts = []
    for ti in range(NT):
        v = pers.tile([128, 130], BF16, tag=f"v{ti}", name=f"v{ti}")
        nc.vector.memset(v[:, 64:65], 1.0)
        nc.vector.memset(v[:, 129:130], 1.0)
        vts.append(v)

    # ---- helpers ----
    def rope_chunk(tgt, tcI, mirror=None):
        """RoPE on one 512-col chunk.  The 32-row-block swap is a PE
        permutation matmul (no DMA -> no queueing behind input loads)."""
        del mirror
        sl = slice(tcI * 512, (tcI + 1) * 512)
        ps = aux.tile([128, 512], F32, tag="pj", name="rsw")
        nc.tensor.matmul(ps[:], p32t[:], tgt[:, sl], start=True, stop=True)
        tmp = rp.tile([128, 512], BF16, tag="tmp", name="tmp")
        nc.vector.tensor_tensor(out=tmp[:], in0=tgt[:, sl], in1=cosr_t[:, sl],
                                op=ALU.mult)
        swp = rp.tile([128, 512], BF16, tag="swp", name="swp")
        nc.vector.tensor_tensor(out=swp[:], in0=ps[:], in1=sins_t[:, sl],
                                op=ALU.mult)
        nc.vector.tensor_tensor(out=tgt[:, sl], in0=tmp[:], in1=swp[:],
                                op=ALU.add)

    def make_quanta(kind, m, tcI):
        """One projection group (16 di-MMs into one [128,512] psum +
        evac), split into 4 quanta of 4 MMs."""
        state = {}

        def quantum(k):
            if k == 0:
                state["ps"] = aux.tile([128, 512], F32, tag="pj", name="pj")
            ps = state["ps"]
            for di in range(4 * k, 4 * k + 4):
                if kind == "q":
                    w = wqTs[di][:, m * 128:(m + 1) * 128]
                elif kind == "k":
                    w = wkTs[di][:]
                else:
                    w = wvTs[di][:]
                nc.tensor.matmul(ps[:], w, xTs[di][tcI][:],
                                 start=(di == 0), stop=(di == ND - 1))
            if k == 3:
                dst = {"q": qts[m] if m is not None else None,
                       "k": kt, "v": vxT}[kind]
                nc.vector.tensor_copy(dst[:, tcI * 512:(tcI + 1) * 512], ps[:])

        return [lambda kk=k: quantum(kk) for k in range(4)]

    def v_tr(ti):
        """PE-transpose one 128-col block of V^T into V' k-tile ti."""
        tp = aux.tile([128, 512], BF16, tag="vtr", name="vtr")
        nc.tensor.transpose(tp[:, 0:128], vxT[:, ti * 128:(ti + 1) * 128],
                            idty[:])
        nc.vector.tensor_copy(vts[ti][:, 0:64], tp[:, 0:64])
        nc.vector.tensor_copy(vts[ti][:, 65:129], tp[:, 64:128])

    # ---- prologue ----
    for f in make_quanta("k", None, 0):
        f()
    rope_chunk(kt, 0)
    for f in make_quanta("q", 0, 0):
        f()
    rope_chunk(qts[0], 0)
    for f in make_quanta("v", None, 0):
        f()
    for ti in range(4):
        v_tr(ti)

    # ---- filler queue ----
    def g_of(j, p):
        return sum(4 * jj + 4 for jj in range(j)) * 4 + (4 * j + 4) * p

    fillers = []

    def add_group(d, kind, m, tcI, rope_tgt=None, mirror=None):
        for f in make_quanta(kind, m, tcI):
            fillers.append((d, f))
        if rope_tgt is not None:
            fillers.append((d, lambda t=rope_tgt, c=tcI, mi=mirror:
                            rope_chunk(t, c, mi)))

    for tcI in range(1, NJ):
        d0 = g_of(tcI, 0)
        add_group(d0 - 8, "k", None, tcI, kt)
        add_group(d0 - 4, "q", 0, tcI, qts[0])
        add_group(d0 + 2, "v", None, tcI)
        for ti in range(4 * tcI, 4 * tcI + 4):
            fillers.append((d0 + 3, lambda t=ti: v_tr(t)))
    for m in range(1, 4):
        for tcI in range(NJ):
            add_group(g_of(tcI, m) - 2, "q", m, tcI, qts[m])
    fillers.sort(key=lambda df: df[0])
    fq = list(fillers)

    # ---- attention stream ----
    items = [(j, p, ki) for j in range(NJ) for p in range(4)
             for ki in range(4 * j + 4)]
    pend = []
    acc = {}
    cur_pt = [None]

    def emit_S(j, p, ki):
        o = max(0, 128 * ki - 512 * j)
        s = sp.tile([128, 1024], F32, tag="s", name="s")
        q0 = 512 * j + o
        q1 = 512 * (j + 1)
        kc = slice(ki * 128, (ki + 1) * 128)
        nc.tensor.matmul(s[:, o:512], kt[0:64, kc],
                         qts[p][0:64, q0:q1], start=True, stop=True)
        nc.tensor.matmul(s[:, 512 + o:1024], kt[64:128, kc],
                         qts[p][64:128, q0:q1], start=True, stop=True)
        if ki >= 4 * j:  # diagonal k-tile: add -3e7 on the q<k triangle
            nc.vector.tensor_tensor(out=s[:, o:o + 128], in0=s[:, o:o + 128],
                                    in1=madd[:], op=ALU.add)
            nc.vector.tensor_tensor(out=s[:, 512 + o:512 + o + 128],
                                    in0=s[:, 512 + o:512 + o + 128],
                                    in1=madd[:], op=ALU.add)
        # pt pair tile: [128, 2048] bf16 = [A_e|A_o|B_e|B_o] planes
        if ki % 2 == 0:
            cur_pt[0] = ptp.tile([128, 2048], BF16, tag="pt", name="pt")
        pt = cur_pt[0]
        src = s[:].rearrange("p (g c) -> p g c", g=2)[:, :, o:512]
        half = pt[:].rearrange("p (g c) -> p g c", g=2)  # halves A / B
        if ki % 2 == 0:
            dst = half[:, :, o:512]
        else:
            dst = half[:, :, 512 + o:1024]
        nc.scalar.activation(dst, src, AF.Exp, scale=SCALE)
        return pt

    def emit_PV(j, p, ki, pt):
        o = max(0, 128 * ki - 512 * j)
        if ki == 0:
            acc[(j, p)] = (po.tile([65, 512], F32, tag="oA", name="oA"),
                           po.tile([65, 512], F32, tag="oB", name="oB"))
        oA, oB = acc[(j, p)]
        last = (ki == 4 * j + 3)
        quads = pt[:].rearrange("p (g c) -> p g c", g=4)
        qa = ki % 2          # plane within pt for this ki
        nc.tensor.matmul(oA[:, o:512], vts[ki][:, 0:65],
                         quads[:, qa, o:512],
                         start=(ki == 0), stop=last)
        nc.tensor.matmul(oB[:, o:512], vts[ki][:, 65:130],
                         quads[:, 2 + qa, o:512],
                         start=(ki == 0), stop=last)
        if last:
            del acc[(j, p)]
            otA = evp.tile([65, 512], F32, tag="oa", name="oa")
            nc.vector.tensor_copy(otA[:], oA[:])
            nc.sync.dma_start(
                out=out[65 * p:65 * (p + 1), 512 * j:512 * (j + 1)],
                in_=otA[:])
            otB = evp.tile([65, 512], F32, tag="ob", name="ob")
            nc.vector.tensor_copy(otB[:], oB[:])
            nc.sync.dma_start(
                out=out[65 * (p + 4):65 * (p + 5), 512 * j:512 * (j + 1)],
                in_=otB[:])

    for g, (j, p, ki) in enumerate(items):
        pt = emit_S(j, p, ki)
        pend.append((j, p, ki, pt))
        if len(pend) > LAG:
            emit_PV(*pend.pop(0))
        emitted = 0
        while fq and (emitted < 1 or fq[0][0] <= g + 2):
            fq.pop(0)[1]()
            emitted += 1
    while pend:
        emit_PV(*pend.pop(0))
    while fq:
        fq.pop(0)[1]()

    ctx.close()


def build_program(T=2048, num_devices=8):
    nc = bacc.Bacc("TRN2", target_bir_lowering=False, debug=False,
                   num_devices=num_devices)
    xT = nc.dram_tensor("xT", (D, T), BF16, kind="ExternalInput").ap()
    wqT = nc.dram_tensor("wqT", (D, 512), BF16, kind="ExternalInput").ap()
    wkT = nc.dram_tensor("wkT", (D, 128), BF16, kind="ExternalInput").ap()
    wvT = nc.dram_tensor("wvT", (D, 128), BF16, kind="ExternalInput").ap()
    cosr = nc.dram_tensor("cosr", (128, T), BF16, kind="ExternalInput").ap()
    sins = nc.dram_tensor("sins", (128, T), BF16, kind="ExternalInput").ap()
    p32 = nc.dram_tensor("p32", (128, 128), BF16, kind="ExternalInput").ap()
    out = nc.dram_tensor("out", (520, T), F32, kind="ExternalOutput").ap()
    with tile.TileContext(nc) as tc:
        _emit_body(tc, (xT, wqT, wkT, wvT, cosr, sins, p32, out), T)
    nc.compile()
    return nc


# ---------------- host side ----------------

def _qperm(j):
    rows = []
    for m in range(4):
        for r in range(128):
            h = m if r < 64 else m + 4
            d = 2 * (r % 32) + (1 if (r % 64) >= 32 else 0)
            rows.append((8 * j + h) * 64 + d)
    return np.array(rows)


def _kperm(j):
    rows = []
    for kv in range(2):
        for r in range(64):
            d = 2 * (r % 32) + (1 if r >= 32 else 0)
            rows.append((2 * j + kv) * 64 + d)
    return np.array(rows)


def make_core_inputs(x, Wq, Wk, Wv, cos, sin):
    """Per-core input dicts (host prep). x: [B,T,D]."""
    bf = ml_dtypes.bfloat16
    B, T, _ = x.shape
    xTb = [np.ascontiguousarray(x[b].T).astype(bf) for b in range(B)]
    cosT = np.ascontiguousarray(cos.T.astype(np.float32))
    sinT = np.ascontiguousarray(sin.T.astype(np.float32))
    cosr = np.tile(cosT, (4, 1)).astype(bf)
    sgn = np.repeat(np.array([-1.0, 1.0, -1.0, 1.0], np.float32), 32)
    sins = (np.tile(sinT, (4, 1)) * sgn[:, None]).astype(bf)
    p32 = np.zeros((128, 128), np.float32)
    swap = np.concatenate([np.arange(32, 64), np.arange(0, 32),
                           np.arange(96, 128), np.arange(64, 96)])
    p32[swap, np.arange(128)] = 1.0
    p32 = p32.astype(bf)
    maps = []
    for c in range(8):
        b, j = c // 4, c % 4
        maps.append({
            "xT": xTb[b],
            "wqT": np.ascontiguousarray(Wq[_qperm(j)].T).astype(bf),
            "wkT": np.ascontiguousarray(Wk[_kperm(j)].T).astype(bf),
            "wvT": np.ascontiguousarray(Wv[128 * j:128 * (j + 1)].T).astype(bf),
            "cosr": cosr,
            "sins": sins,
            "p32": p32,
        })
    return maps


def unshard(results, B=2, T=2048):
    """Combine per-core O^T blocks into the full [B, T, 2048] output."""
    out = np.empty((B, T, 2048), np.float32)
    for c in range(8):
        b, j = c // 4, c % 4
        blk = results[c]["out"].reshape(8, 65, T)
        O = blk[:, :64, :] / blk[:, 64:65, :]
        out[b, :, 512 * j:512 * (j + 1)] = (
            O.transpose(2, 0, 1).reshape(T, 512))
    return out


_CACHE = {}


def _get_program():
    if "nc" not in _CACHE:
        _CACHE["nc"] = build_program(T=2048, num_devices=8)
    return _CACHE["nc"]


def run_on_hw(in_maps, trace=False, tmpdir=None):
    nc = _get_program()
    return run_bass_kernel_spmd(nc, in_maps, list(range(8)), trace=trace,
                                tmpdir=tmpdir)


def kernel(x, Wq, Wk, Wv, cos, sin):
    x = np.asarray(x, np.float32)
    Wq = np.asarray(Wq, np.float32)
    Wk = np.asarray(Wk, np.float32)
    Wv = np.asarray(Wv, np.float32)
    cos = np.asarray(cos, np.float32)
    sin = np.asarray(sin, np.float32)
    maps = make_core_inputs(x, Wq, Wk, Wv, cos, sin)
    res = run_on_hw(maps, trace=False)
    return unshard(res.results, x.shape[0], x.shape[1])


# revision 32
# speedup vs baseline: 1.1794x; 1.0148x over previous
"""GQA kernel for Trainium2 (Bass/Tile), 8 NeuronCores — v4.

Sharding: core c -> batch b=c//4, kv-head pair j=c%4 (kv heads 2j,2j+1,
q heads 8j..8j+7).  Each core computes out[b, :, 512j:512(j+1)].

Structure (all compute bf16 / fp32-accum; fp8 was tested and rejected —
QK-fp8 alone costs 3.8e-2 rel err, P-fp8 5.5e-2, both over budget):
  * q-chunk-major attention: for j (512-wide q chunk) -> for pair
    (head m & m+4) -> for ki (k-tile): S^T chunk-pair, exp, PV.
  * S^T chunk-pair via row-tiled CONCURRENT matmuls: head m uses
    kt/qt partitions 0:64 (tile_position (0,0)), head m+4 uses 64:128
    ((64,0)) -> full PE array despite K=64 contraction.
  * causal mask: DVE adds -3e7 onto the diagonal 128x128 S^T block in
    PSUM before exp (exp then gives exact 0).
  * one exp ACTIVATE per item covers both heads ([128,2,w] strided AP
    over the 2-bank S pair); output bf16 into quad-plane pt tiles.
  * PV with V' stationary (V k-tile 64 cols + ones column, M=65),
    P^T chunk moving (N<=512) -> O^T[d|den, q] accumulates in PSUM
    over ki.  Stream-bound instead of LDWEIGHTS-bound.
  * V computed as V^T (wv stationary, x moving; stream-efficient),
    then PE-transposed per 128-col block into V' k-tiles.
  * output stays in O^T layout [8 heads x 65, T]; host divides by the
    denominator row and transposes.
  * projections split into 512-col chunk groups, woven through the
    attention stream as deadline-ordered fillers; input DMA ordered
    wk -> x.tc0 -> wq -> x rest -> wv across both HWDGE queues.
"""

import sys

for _p in ("/opt/trn_rl_repo",):
    if _p not in sys.path:
        sys.path.insert(0, _p)

import numpy as np
import ml_dtypes

import concourse.bass as bass
import concourse.tile as tile
from concourse import bacc, mybir
from concourse.bass_utils import run_bass_kernel_spmd
from concourse.masks import make_lower_triangular, make_identity

BF16 = mybir.dt.bfloat16
F32 = mybir.dt.float32
AF = mybir.ActivationFunctionType
ALU = mybir.AluOpType

D = 2048
HS = 64
SCALE = 0.125       # 1/sqrt(HS)
LAG = 3             # S->PV software-pipeline lag (items)


def _emit_body(tc, aps, T):
    nc = tc.nc
    NT = T // 128            # k tiles
    NJ = T // 512            # q chunks
    ND = D // 128            # contraction chunks

    xT, wqT, wkT, wvT, cosr, sins, p32, out = aps

    import contextlib
    ctx = tc._kernel_exitstack = contextlib.ExitStack()

    pers = ctx.enter_context(tc.tile_pool(name="pers", bufs=1))
    rp = ctx.enter_context(tc.tile_pool(name="rope", bufs=2))
    ptp = ctx.enter_context(tc.tile_pool(name="ptp", bufs=4))
    evp = ctx.enter_context(tc.tile_pool(name="evp", bufs=2))
    sp = ctx.enter_context(tc.tile_pool(name="spsum", bufs=2, space="PSUM"))
    po = ctx.enter_context(tc.tile_pool(name="opsum", bufs=1, space="PSUM"))
    aux = ctx.enter_context(tc.tile_pool(name="aux", bufs=1, space="PSUM"))

    # ---- input DMA: both HWDGE queues; priority order so K/Q0 start early
    wqTs, wkTs, wvTs = [], [], []
    xTs = [[None] * NJ for _ in range(ND)]
    for di in range(ND):
        t = pers.tile([128, 128], BF16, tag=f"wk{di}", name=f"wk{di}")
        nc.sync.dma_start(out=t[:], in_=wkT[di * 128:(di + 1) * 128, :])
        wkTs.append(t)
    for tcI in range(NJ):
        for di in range(ND):
            xTs[di][tcI] = pers.tile([128, 512], BF16, tag=f"x{di}_{tcI}",
                                     name=f"x{di}_{tcI}")

    def dma_x(q, di, tcI):
        eng = nc.sync if q == 0 else nc.gpsimd
        eng.dma_start(out=xTs[di][tcI][:],
                      in_=xT[di * 128:(di + 1) * 128,
                             tcI * 512:(tcI + 1) * 512])

    for di in range(ND):
        wqTs.append(pers.tile([128, 512], BF16, tag=f"wq{di}",
                              name=f"wq{di}"))
        wvTs.append(pers.tile([128, 128], BF16, tag=f"wv{di}",
                              name=f"wv{di}"))
    # sync (HWDGE): cos/sin chunk0 -> x.tc0 -> wq m0 -> wv -> xe.tc1 ->
    #               wq m1-3 -> xe.tc2-3.
    # gpsimd (SWDGE, slower, late-needed only): cos/sin rest, xo.tc1-3.
    cosr_t = pers.tile([128, T], BF16, tag="cosr", name="cosr")
    sins_t = pers.tile([128, T], BF16, tag="sins", name="sins")
    p32t = pers.tile([128, 128], BF16, tag="p32", name="p32")
    nc.sync.dma_start(out=p32t[:], in_=p32[:, :])
    nc.sync.dma_start(out=cosr_t[:, 0:512], in_=cosr[:, 0:512])
    nc.sync.dma_start(out=sins_t[:, 0:512], in_=sins[:, 0:512])
    nc.gpsimd.dma_start(out=cosr_t[:, 512:T], in_=cosr[:, 512:T])
    nc.gpsimd.dma_start(out=sins_t[:, 512:T], in_=sins[:, 512:T])
    for di in range(0, ND, 2):
        dma_x(0, di, 0)
    for di in range(1, ND, 2):
        dma_x(1, di, 0)
    for di in range(ND):
        nc.sync.dma_start(out=wqTs[di][:, 0:128],
                          in_=wqT[di * 128:(di + 1) * 128, 0:128])
    for di in range(ND):
        nc.sync.dma_start(out=wvTs[di][:],
                          in_=wvT[di * 128:(di + 1) * 128, :])
    for di in range(0, ND, 2):
        dma_x(0, di, 1)
    for di in range(1, ND, 2):
        dma_x(1, di, 1)
    for di in range(ND):
        nc.sync.dma_start(out=wqTs[di][:, 128:512],
                          in_=wqT[di * 128:(di + 1) * 128, 128:512])
    for tcI in range(2, NJ):
        for di in range(0, ND, 2):
            dma_x(0, di, tcI)
        for di in range(1, ND, 2):
            dma_x(1, di, tcI)

    madd = pers.tile([128, 128], F32, tag="madd", name="madd")
    make_lower_triangular(nc, madd[:], val=-3.0e7, diag=False)
    idty = pers.tile([128, 128], BF16, tag="idty", name="idty")
    make_identity(nc, idty[:])

    qts = [pers.tile([128, T], BF16, tag=f"qt{m}", name=f"qt{m}")
           for m in range(4)]
    kt = pers.tile([128, T], BF16, tag="kt", name="kt")
    vxT = pers.tile([128, T], BF16, tag="vxT", name="vxT")
    vts = []
    for ti in range(NT):
        v = pers.tile([128, 130], BF16, tag=f"v{ti}", name=f"v{ti}")
        nc.vector.memset(v[:, 64:65], 1.0)
        nc.vector.memset(v[:, 129:130], 1.0)
        vts.append(v)

    # ---- helpers ----
    def rope_chunk(tgt, tcI, mirror=None):
        """RoPE on one 512-col chunk.  The 32-row-block swap is a PE
        permutation matmul (no DMA -> no queueing behind input loads)."""
        del mirror
        sl = slice(tcI * 512, (tcI + 1) * 512)
        ps = aux.tile([128, 512], F32, tag="pj", name="rsw")
        nc.tensor.matmul(ps[:], p32t[:], tgt[:, sl], start=True, stop=True)
        tmp = rp.tile([128, 512], BF16, tag="tmp", name="tmp")
        nc.vector.tensor_tensor(out=tmp[:], in0=tgt[:, sl], in1=cosr_t[:, sl],
                                op=ALU.mult)
        swp = rp.tile([128, 512], BF16, tag="swp", name="swp")
        nc.vector.tensor_tensor(out=swp[:], in0=ps[:], in1=sins_t[:, sl],
                                op=ALU.mult)
        nc.vector.tensor_tensor(out=tgt[:, sl], in0=tmp[:], in1=swp[:],
                                op=ALU.add)

    def make_quanta(kind, m, tcI):
        """One projection group (16 di-MMs into one [128,512] psum +
        evac), split into 4 quanta of 4 MMs."""
        state = {}

        def quantum(k):
            if k == 0:
                state["ps"] = aux.tile([128, 512], F32, tag="pj", name="pj")
            ps = state["ps"]
            for di in range(4 * k, 4 * k + 4):
                if kind == "q":
                    w = wqTs[di][:, m * 128:(m + 1) * 128]
                elif kind == "k":
                    w = wkTs[di][:]
                else:
                    w = wvTs[di][:]
                nc.tensor.matmul(ps[:], w, xTs[di][tcI][:],
                                 start=(di == 0), stop=(di == ND - 1))
            if k == 3:
                dst = {"q": qts[m] if m is not None else None,
                       "k": kt, "v": vxT}[kind]
                nc.vector.tensor_copy(dst[:, tcI * 512:(tcI + 1) * 512], ps[:])

        return [lambda kk=k: quantum(kk) for k in range(4)]

    def v_tr(ti):
        """PE-transpose one 128-col block of V^T into V' k-tile ti."""
        tp = aux.tile([128, 512], BF16, tag="vtr", name="vtr")
        nc.tensor.transpose(tp[:, 0:128], vxT[:, ti * 128:(ti + 1) * 128],
                            idty[:])
        nc.vector.tensor_copy(vts[ti][:, 0:64], tp[:, 0:64])
        nc.vector.tensor_copy(vts[ti][:, 65:129], tp[:, 64:128])

    # ---- prologue ----
    for f in make_quanta("k", None, 0):
        f()
    rope_chunk(kt, 0)
    for f in make_quanta("q", 0, 0):
        f()
    rope_chunk(qts[0], 0)
    for f in make_quanta("v", None, 0):
        f()
    for ti in range(4):
        v_tr(ti)

    # ---- filler queue ----
    def g_of(j, p):
        return sum(4 * jj + 4 for jj in range(j)) * 4 + (4 * j + 4) * p

    fillers = []

    def add_group(d, kind, m, tcI, rope_tgt=None, mirror=None):
        for f in make_quanta(kind, m, tcI):
            fillers.append((d, f))
        if rope_tgt is not None:
            fillers.append((d, lambda t=rope_tgt, c=tcI, mi=mirror:
                            rope_chunk(t, c, mi)))

    for tcI in range(1, NJ):
        d0 = g_of(tcI, 0)
        add_group(d0 - 8, "k", None, tcI, kt)
        add_group(d0 - 4, "q", 0, tcI, qts[0])
        add_group(d0 + 2, "v", None, tcI)
        for ti in range(4 * tcI, 4 * tcI + 4):
            fillers.append((d0 + 3, lambda t=ti: v_tr(t)))
    for m in range(1, 4):
        for tcI in range(NJ):
            add_group(g_of(tcI, m) - 2, "q", m, tcI, qts[m])
    fillers.sort(key=lambda df: df[0])
    fq = list(fillers)

    # ---- attention stream ----
    items = [(j, p, ki) for j in range(NJ) for p in range(4)
             for ki in range(4 * j + 4)]
    pend = []
    acc = {}
    cur_pt = [None]

    def emit_S(j, p, ki):
        o = max(0, 128 * ki - 512 * j)
        s = sp.tile([128, 1024], F32, tag="s", name="s")
        q0 = 512 * j + o
        q1 = 512 * (j + 1)
        kc = slice(ki * 128, (ki + 1) * 128)
        nc.tensor.matmul(s[:, o:512], kt[0:64, kc],
                         qts[p][0:64, q0:q1], start=True, stop=True)
        nc.tensor.matmul(s[:, 512 + o:1024], kt[64:128, kc],
                         qts[p][64:128, q0:q1], start=True, stop=True)
        if ki >= 4 * j:  # diagonal k-tile: add -3e7 on the q<k triangle
            nc.vector.tensor_tensor(out=s[:, o:o + 128], in0=s[:, o:o + 128],
                                    in1=madd[:], op=ALU.add)
            nc.vector.tensor_tensor(out=s[:, 512 + o:512 + o + 128],
                                    in0=s[:, 512 + o:512 + o + 128],
                                    in1=madd[:], op=ALU.add)
        # pt pair tile: [128, 2048] bf16 = [A_e|A_o|B_e|B_o] planes
        if ki % 2 == 0:
            cur_pt[0] = ptp.tile([128, 2048], BF16, tag="pt", name="pt")
        pt = cur_pt[0]
        src = s[:].rearrange("p (g c) -> p g c", g=2)[:, :, o:512]
        half = pt[:].rearrange("p (g c) -> p g c", g=2)  # halves A / B
        if ki % 2 == 0:
            dst = half[:, :, o:512]
        else:
            dst = half[:, :, 512 + o:1024]
        nc.scalar.activation(dst, src, AF.Exp, scale=SCALE)
        return pt

    def emit_PV(j, p, ki, pt):
        o = max(0, 128 * ki - 512 * j)
        if ki == 0:
            acc[(j, p)] = (po.tile([65, 512], F32, tag="oA", name="oA"),
                           po.tile([65, 512], F32, tag="oB", name="oB"))
        oA, oB = acc[(j, p)]
        last = (ki == 4 * j + 3)
        quads = pt[:].rearrange("p (g c) -> p g c", g=4)
        qa = ki % 2          # plane within pt for this ki
        nc.tensor.matmul(oA[:, o:512], vts[ki][:, 0:65],
                         quads[:, qa, o:512],
                         start=(ki == 0), stop=last)
        nc.tensor.matmul(oB[:, o:512], vts[ki][:, 65:130],
                         quads[:, 2 + qa, o:512],
                         start=(ki == 0), stop=last)
        if last:
            del acc[(j, p)]
            otA = evp.tile([65, 512], F32, tag="oa", name="oa")
            nc.vector.tensor_copy(otA[:], oA[:])
            nc.sync.dma_start(
                out=out[65 * p:65 * (p + 1), 512 * j:512 * (j + 1)],
                in_=otA[:])
            otB = evp.tile([65, 512], F32, tag="ob", name="ob")
            nc.vector.tensor_copy(otB[:], oB[:])
            nc.sync.dma_start(
                out=out[65 * (p + 4):65 * (p + 5), 512 * j:512 * (j + 1)],
                in_=otB[:])

    for g, (j, p, ki) in enumerate(items):
        pt = emit_S(j, p, ki)
        pend.append((j, p, ki, pt))
        if len(pend) > LAG:
            emit_PV(*pend.pop(0))
        emitted = 0
        while fq and (emitted < 1 or fq[0][0] <= g + 2):
            fq.pop(0)[1]()
            emitted += 1
    while pend:
        emit_PV(*pend.pop(0))
    while fq:
        fq.pop(0)[1]()

    ctx.close()


def build_program(T=2048, num_devices=8):
    nc = bacc.Bacc("TRN2", target_bir_lowering=False, debug=False,
                   num_devices=num_devices)
    xT = nc.dram_tensor("xT", (D, T), BF16, kind="ExternalInput").ap()
    wqT = nc.dram_tensor("wqT", (D, 512), BF16, kind="ExternalInput").ap()
    wkT = nc.dram_tensor("wkT", (D, 128), BF16, kind="ExternalInput").ap()
    wvT = nc.dram_tensor("wvT", (D, 128), BF16, kind="ExternalInput").ap()
    cosr = nc.dram_tensor("cosr", (128, T), BF16, kind="ExternalInput").ap()
    sins = nc.dram_tensor("sins", (128, T), BF16, kind="ExternalInput").ap()
    p32 = nc.dram_tensor("p32", (128, 128), BF16, kind="ExternalInput").ap()
    out = nc.dram_tensor("out", (520, T), F32, kind="ExternalOutput").ap()
    with tile.TileContext(nc) as tc:
        _emit_body(tc, (xT, wqT, wkT, wvT, cosr, sins, p32, out), T)
    nc.compile()
    return nc


# ---------------- host side ----------------

def _qperm(j):
    rows = []
    for m in range(4):
        for r in range(128):
            h = m if r < 64 else m + 4
            d = 2 * (r % 32) + (1 if (r % 64) >= 32 else 0)
            rows.append((8 * j + h) * 64 + d)
    return np.array(rows)


def _kperm(j):
    rows = []
    for kv in range(2):
        for r in range(64):
            d = 2 * (r % 32) + (1 if r >= 32 else 0)
            rows.append((2 * j + kv) * 64 + d)
    return np.array(rows)


def make_core_inputs(x, Wq, Wk, Wv, cos, sin):
    """Per-core input dicts (host prep). x: [B,T,D]."""
    bf = ml_dtypes.bfloat16
    B, T, _ = x.shape
    xTb = [np.ascontiguousarray(x[b].T).astype(bf) for b in range(B)]
    cosT = np.ascontiguousarray(cos.T.astype(np.float32))
    sinT = np.ascontiguousarray(sin.T.astype(np.float32))
    cosr = np.tile(cosT, (4, 1)).astype(bf)
    sgn = np.repeat(np.array([-1.0, 1.0, -1.0, 1.0], np.float32), 32)
    sins = (np.tile(sinT, (4, 1)) * sgn[:, None]).astype(bf)
    p32 = np.zeros((128, 128), np.float32)
    swap = np.concatenate([np.arange(32, 64), np.arange(0, 32),
                           np.arange(96, 128), np.arange(64, 96)])
    p32[swap, np.arange(128)] = 1.0
    p32 = p32.astype(bf)
    maps = []
    for c in range(8):
        b, j = c // 4, c % 4
        maps.append({
            "xT": xTb[b],
            "wqT": np.ascontiguousarray(Wq[_qperm(j)].T).astype(bf),
            "wkT": np.ascontiguousarray(Wk[_kperm(j)].T).astype(bf),
            "wvT": np.ascontiguousarray(Wv[128 * j:128 * (j + 1)].T).astype(bf),
            "cosr": cosr,
            "sins": sins,
            "p32": p32,
        })
    return maps


def unshard(results, B=2, T=2048):
    """Combine per-core O^T blocks into the full [B, T, 2048] output."""
    out = np.empty((B, T, 2048), np.float32)
    for c in range(8):
        b, j = c // 4, c % 4
        blk = results[c]["out"].reshape(8, 65, T)
        O = blk[:, :64, :] / blk[:, 64:65, :]
        out[b, :, 512 * j:512 * (j + 1)] = (
            O.transpose(2, 0, 1).reshape(T, 512))
    return out


_CACHE = {}


def _get_program():
    if "nc" not in _CACHE:
        _CACHE["nc"] = build_program(T=2048, num_devices=8)
    return _CACHE["nc"]


def run_on_hw(in_maps, trace=False, tmpdir=None):
    nc = _get_program()
    return run_bass_kernel_spmd(nc, in_maps, list(range(8)), trace=trace,
                                tmpdir=tmpdir)


def kernel(x, Wq, Wk, Wv, cos, sin):
    x = np.asarray(x, np.float32)
    Wq = np.asarray(Wq, np.float32)
    Wk = np.asarray(Wk, np.float32)
    Wv = np.asarray(Wv, np.float32)
    cos = np.asarray(cos, np.float32)
    sin = np.asarray(sin, np.float32)
    maps = make_core_inputs(x, Wq, Wk, Wv, cos, sin)
    res = run_on_hw(maps, trace=False)
    return unshard(res.results, x.shape[0], x.shape[1])


# revision 34
# speedup vs baseline: 1.2071x; 1.0235x over previous
"""GQA kernel for Trainium2 (Bass/Tile), 8 NeuronCores — v4.

Sharding: core c -> batch b=c//4, kv-head pair j=c%4 (kv heads 2j,2j+1,
q heads 8j..8j+7).  Each core computes out[b, :, 512j:512(j+1)].

Structure (all compute bf16 / fp32-accum; fp8 was tested and rejected —
QK-fp8 alone costs 3.8e-2 rel err, P-fp8 5.5e-2, both over budget):
  * q-chunk-major attention: for j (512-wide q chunk) -> for pair
    (head m & m+4) -> for ki (k-tile): S^T chunk-pair, exp, PV.
  * S^T chunk-pair via row-tiled CONCURRENT matmuls: head m uses
    kt/qt partitions 0:64 (tile_position (0,0)), head m+4 uses 64:128
    ((64,0)) -> full PE array despite K=64 contraction.
  * causal mask: DVE adds -3e7 onto the diagonal 128x128 S^T block in
    PSUM before exp (exp then gives exact 0).
  * one exp ACTIVATE per item covers both heads ([128,2,w] strided AP
    over the 2-bank S pair); output bf16 into quad-plane pt tiles.
  * PV with V' stationary (V k-tile 64 cols + ones column, M=65),
    P^T chunk moving (N<=512) -> O^T[d|den, q] accumulates in PSUM
    over ki.  Stream-bound instead of LDWEIGHTS-bound.
  * V computed as V^T (wv stationary, x moving; stream-efficient),
    then PE-transposed per 128-col block into V' k-tiles.
  * output stays in O^T layout [8 heads x 65, T]; host divides by the
    denominator row and transposes.
  * projections split into 512-col chunk groups, woven through the
    attention stream as deadline-ordered fillers; input DMA ordered
    wk -> x.tc0 -> wq -> x rest -> wv across both HWDGE queues.
"""

import sys

for _p in ("/opt/trn_rl_repo",):
    if _p not in sys.path:
        sys.path.insert(0, _p)

import numpy as np
import ml_dtypes

import concourse.bass as bass
import concourse.tile as tile
from concourse import bacc, mybir
from concourse.bass_utils import run_bass_kernel_spmd
from concourse.masks import make_lower_triangular, make_identity

BF16 = mybir.dt.bfloat16
F32 = mybir.dt.float32
AF = mybir.ActivationFunctionType
ALU = mybir.AluOpType

D = 2048
HS = 64
SCALE = 0.125       # 1/sqrt(HS)
LAG = 3             # S->PV software-pipeline lag (items)


def _emit_body(tc, aps, T):
    nc = tc.nc
    NT = T // 128            # k tiles
    NJ = T // 512            # q chunks
    ND = D // 128            # contraction chunks

    xT, wqT, wkT, wvT, cosr, sins, p32, out = aps

    import contextlib
    ctx = tc._kernel_exitstack = contextlib.ExitStack()

    pers = ctx.enter_context(tc.tile_pool(name="pers", bufs=1))
    rp = ctx.enter_context(tc.tile_pool(name="rope", bufs=2))
    ptp = ctx.enter_context(tc.tile_pool(name="ptp", bufs=4))
    evp = ctx.enter_context(tc.tile_pool(name="evp", bufs=2))
    sp = ctx.enter_context(tc.tile_pool(name="spsum", bufs=2, space="PSUM"))
    po = ctx.enter_context(tc.tile_pool(name="opsum", bufs=1, space="PSUM"))
    aux = ctx.enter_context(tc.tile_pool(name="aux", bufs=1, space="PSUM"))

    # ---- input DMA: both HWDGE queues; priority order so K/Q0 start early
    wqTs, wkTs, wvTs = [], [], []
    xTs = [[None] * NJ for _ in range(ND)]
    for di in range(ND):
        t = pers.tile([128, 128], BF16, tag=f"wk{di}", name=f"wk{di}")
        nc.sync.dma_start(out=t[:], in_=wkT[di * 128:(di + 1) * 128, :])
        wkTs.append(t)
    for tcI in range(NJ):
        for di in range(ND):
            xTs[di][tcI] = pers.tile([128, 512], BF16, tag=f"x{di}_{tcI}",
                                     name=f"x{di}_{tcI}")

    def dma_x(q, di, tcI):
        eng = nc.sync if q == 0 else nc.gpsimd
        eng.dma_start(out=xTs[di][tcI][:],
                      in_=xT[di * 128:(di + 1) * 128,
                             tcI * 512:(tcI + 1) * 512])

    for di in range(ND):
        wqTs.append(pers.tile([128, 512], BF16, tag=f"wq{di}",
                              name=f"wq{di}"))
        wvTs.append(pers.tile([128, 128], BF16, tag=f"wv{di}",
                              name=f"wv{di}"))
    # sync (HWDGE): cos/sin chunk0 -> x.tc0 -> wq m0 -> wv -> xe.tc1 ->
    #               wq m1-3 -> xe.tc2-3.
    # gpsimd (SWDGE, slower, late-needed only): cos/sin rest, xo.tc1-3.
    cosr_t = pers.tile([128, T], BF16, tag="cosr", name="cosr")
    sins_t = pers.tile([128, T], BF16, tag="sins", name="sins")
    p32t = pers.tile([128, 128], BF16, tag="p32", name="p32")
    nc.sync.dma_start(out=p32t[:], in_=p32[:, :])
    nc.sync.dma_start(out=cosr_t[:, 0:512], in_=cosr[:, 0:512])
    nc.sync.dma_start(out=sins_t[:, 0:512], in_=sins[:, 0:512])
    nc.gpsimd.dma_start(out=cosr_t[:, 512:T], in_=cosr[:, 512:T])
    nc.gpsimd.dma_start(out=sins_t[:, 512:T], in_=sins[:, 512:T])
    for di in range(0, ND, 2):
        dma_x(0, di, 0)
    for di in range(1, ND, 2):
        dma_x(1, di, 0)
    for di in range(ND):
        nc.sync.dma_start(out=wqTs[di][:, 0:128],
                          in_=wqT[di * 128:(di + 1) * 128, 0:128])
    for di in range(ND):
        nc.sync.dma_start(out=wvTs[di][:],
                          in_=wvT[di * 128:(di + 1) * 128, :])
    for di in range(0, ND, 2):
        dma_x(0, di, 1)
    for di in range(1, ND, 2):
        dma_x(1, di, 1)
    for di in range(ND):
        nc.sync.dma_start(out=wqTs[di][:, 128:512],
                          in_=wqT[di * 128:(di + 1) * 128, 128:512])
    for tcI in range(2, NJ):
        for di in range(0, ND, 2):
            dma_x(0, di, tcI)
        for di in range(1, ND, 2):
            dma_x(1, di, tcI)

    madd = pers.tile([128, 128], F32, tag="madd", name="madd")
    make_lower_triangular(nc, madd[:], val=-3.0e7, diag=False)
    idty = pers.tile([128, 128], BF16, tag="idty", name="idty")
    make_identity(nc, idty[:])

    qts = [pers.tile([128, T], BF16, tag=f"qt{m}", name=f"qt{m}")
           for m in range(4)]
    kt = pers.tile([128, T], BF16, tag="kt", name="kt")
    vxT = pers.tile([128, T], BF16, tag="vxT", name="vxT")
    vts = []
    for ti in range(NT):
        v = pers.tile([128, 130], BF16, tag=f"v{ti}", name=f"v{ti}")
        nc.vector.memset(v[:, 64:65], 1.0)
        nc.vector.memset(v[:, 129:130], 1.0)
        vts.append(v)

    # ---- helpers ----
    def rope_chunk(tgt, tcI, mirror=None):
        """RoPE on one 512-col chunk.  The 32-row-block swap is a PE
        permutation matmul (no DMA -> no queueing behind input loads)."""
        del mirror
        sl = slice(tcI * 512, (tcI + 1) * 512)
        ps = aux.tile([128, 512], F32, tag="pj", name="rsw")
        nc.tensor.matmul(ps[:], p32t[:], tgt[:, sl], start=True, stop=True)
        tmp = rp.tile([128, 512], BF16, tag="tmp", name="tmp")
        nc.vector.tensor_tensor(out=tmp[:], in0=tgt[:, sl], in1=cosr_t[:, sl],
                                op=ALU.mult)
        swp = rp.tile([128, 512], BF16, tag="swp", name="swp")
        nc.vector.tensor_tensor(out=swp[:], in0=ps[:], in1=sins_t[:, sl],
                                op=ALU.mult)
        nc.vector.tensor_tensor(out=tgt[:, sl], in0=tmp[:], in1=swp[:],
                                op=ALU.add)

    def make_quanta(kind, m, tcI):
        """One projection group (16 di-MMs into one [128,512] psum +
        evac), split into 4 quanta of 4 MMs."""
        state = {}

        def quantum(k):
            if k == 0:
                state["ps"] = aux.tile([128, 512], F32, tag="pj", name="pj")
            ps = state["ps"]
            for di in range(4 * k, 4 * k + 4):
                if kind == "q":
                    w = wqTs[di][:, m * 128:(m + 1) * 128]
                elif kind == "k":
                    w = wkTs[di][:]
                else:
                    w = wvTs[di][:]
                nc.tensor.matmul(ps[:], w, xTs[di][tcI][:],
                                 start=(di == 0), stop=(di == ND - 1))
            if k == 3:
                dst = {"q": qts[m] if m is not None else None,
                       "k": kt, "v": vxT}[kind]
                nc.vector.tensor_copy(dst[:, tcI * 512:(tcI + 1) * 512], ps[:])

        return [lambda kk=k: quantum(kk) for k in range(4)]

    def v_tr(ti):
        """PE-transpose one 128-col block of V^T into V' k-tile ti."""
        tp = aux.tile([128, 512], BF16, tag="vtr", name="vtr")
        nc.tensor.transpose(tp[:, 0:128], vxT[:, ti * 128:(ti + 1) * 128],
                            idty[:])
        nc.vector.tensor_copy(vts[ti][:, 0:64], tp[:, 0:64])
        nc.vector.tensor_copy(vts[ti][:, 65:129], tp[:, 64:128])

    # ---- prologue ----
    for f in make_quanta("k", None, 0):
        f()
    rope_chunk(kt, 0)
    for f in make_quanta("q", 0, 0):
        f()
    rope_chunk(qts[0], 0)
    for f in make_quanta("v", None, 0):
        f()
    for ti in range(4):
        v_tr(ti)

    # ---- filler queue ----
    def g_of(j, p):
        return sum(4 * jj + 4 for jj in range(j)) * 4 + (4 * j + 4) * p

    fillers = []

    def add_group(d, kind, m, tcI, rope_tgt=None, mirror=None):
        for f in make_quanta(kind, m, tcI):
            fillers.append((d, f))
        if rope_tgt is not None:
            fillers.append((d, lambda t=rope_tgt, c=tcI, mi=mirror:
                            rope_chunk(t, c, mi)))

    # arrival estimates (in items) for x chunk tcI: emitting a quantum
    # before its x DMA lands head-of-line-blocks the PE FIFO.
    arr = {0: 0, 1: 14, 2: 26, 3: 38}
    for tcI in range(1, NJ):
        d0 = g_of(tcI, 0)
        add_group(max(d0 - 8, arr[tcI]), "k", None, tcI, kt)
        add_group(max(d0 - 4, arr[tcI] + 1), "q", 0, tcI, qts[0])
        add_group(max(d0 + 2, arr[tcI] + 2), "v", None, tcI)
        for ti in range(4 * tcI, 4 * tcI + 4):
            fillers.append((max(d0 + 3, arr[tcI] + 3), lambda t=ti: v_tr(t)))
    for m in range(1, 4):
        for tcI in range(NJ):
            add_group(max(g_of(tcI, m) - 2, arr[tcI]), "q", m, tcI, qts[m])
    fillers.sort(key=lambda df: df[0])
    fq = list(fillers)

    # ---- attention stream ----
    items = [(j, p, ki) for j in range(NJ) for p in range(4)
             for ki in range(4 * j + 4)]
    pend = []
    acc = {}
    cur_pt = [None]

    def emit_S(j, p, ki):
        o = max(0, 128 * ki - 512 * j)
        s = sp.tile([128, 1024], F32, tag="s", name="s")
        q0 = 512 * j + o
        q1 = 512 * (j + 1)
        kc = slice(ki * 128, (ki + 1) * 128)
        nc.tensor.matmul(s[:, o:512], kt[0:64, kc],
                         qts[p][0:64, q0:q1], start=True, stop=True)
        nc.tensor.matmul(s[:, 512 + o:1024], kt[64:128, kc],
                         qts[p][64:128, q0:q1], start=True, stop=True)
        if ki >= 4 * j:  # diagonal k-tile: add -3e7 on the q<k triangle
            nc.vector.tensor_tensor(out=s[:, o:o + 128], in0=s[:, o:o + 128],
                                    in1=madd[:], op=ALU.add)
            nc.vector.tensor_tensor(out=s[:, 512 + o:512 + o + 128],
                                    in0=s[:, 512 + o:512 + o + 128],
                                    in1=madd[:], op=ALU.add)
        # pt pair tile: [128, 2048] bf16 = [A_e|A_o|B_e|B_o] planes
        if ki % 2 == 0:
            cur_pt[0] = ptp.tile([128, 2048], BF16, tag="pt", name="pt")
        pt = cur_pt[0]
        src = s[:].rearrange("p (g c) -> p g c", g=2)[:, :, o:512]
        half = pt[:].rearrange("p (g c) -> p g c", g=2)  # halves A / B
        if ki % 2 == 0:
            dst = half[:, :, o:512]
        else:
            dst = half[:, :, 512 + o:1024]
        nc.scalar.activation(dst, src, AF.Exp, scale=SCALE)
        return pt

    def emit_PV(j, p, ki, pt):
        o = max(0, 128 * ki - 512 * j)
        if ki == 0:
            acc[(j, p)] = (po.tile([65, 512], F32, tag="oA", name="oA"),
                           po.tile([65, 512], F32, tag="oB", name="oB"))
        oA, oB = acc[(j, p)]
        last = (ki == 4 * j + 3)
        quads = pt[:].rearrange("p (g c) -> p g c", g=4)
        qa = ki % 2          # plane within pt for this ki
        nc.tensor.matmul(oA[:, o:512], vts[ki][:, 0:65],
                         quads[:, qa, o:512],
                         start=(ki == 0), stop=last)
        nc.tensor.matmul(oB[:, o:512], vts[ki][:, 65:130],
                         quads[:, 2 + qa, o:512],
                         start=(ki == 0), stop=last)
        if last:
            del acc[(j, p)]
            otA = evp.tile([65, 512], F32, tag="oa", name="oa")
            nc.vector.tensor_copy(otA[:], oA[:])
            nc.sync.dma_start(
                out=out[65 * p:65 * (p + 1), 512 * j:512 * (j + 1)],
                in_=otA[:])
            otB = evp.tile([65, 512], F32, tag="ob", name="ob")
            nc.vector.tensor_copy(otB[:], oB[:])
            nc.sync.dma_start(
                out=out[65 * (p + 4):65 * (p + 5), 512 * j:512 * (j + 1)],
                in_=otB[:])

    for g, (j, p, ki) in enumerate(items):
        pt = emit_S(j, p, ki)
        pend.append((j, p, ki, pt))
        if len(pend) > LAG:
            emit_PV(*pend.pop(0))
        # emit fillers: force anything due, spread the rest one per item
        # but never look further ahead than ~6 items (premature emission
        # of a quantum whose x chunk hasn't landed stalls the PE FIFO).
        emitted = 0
        while fq and (fq[0][0] <= g + 2 or
                      (emitted < 1 and fq[0][0] <= g + 6)):
            fq.pop(0)[1]()
            emitted += 1
    while pend:
        emit_PV(*pend.pop(0))
    while fq:
        fq.pop(0)[1]()

    ctx.close()


def build_program(T=2048, num_devices=8):
    nc = bacc.Bacc("TRN2", target_bir_lowering=False, debug=False,
                   num_devices=num_devices)
    xT = nc.dram_tensor("xT", (D, T), BF16, kind="ExternalInput").ap()
    wqT = nc.dram_tensor("wqT", (D, 512), BF16, kind="ExternalInput").ap()
    wkT = nc.dram_tensor("wkT", (D, 128), BF16, kind="ExternalInput").ap()
    wvT = nc.dram_tensor("wvT", (D, 128), BF16, kind="ExternalInput").ap()
    cosr = nc.dram_tensor("cosr", (128, T), BF16, kind="ExternalInput").ap()
    sins = nc.dram_tensor("sins", (128, T), BF16, kind="ExternalInput").ap()
    p32 = nc.dram_tensor("p32", (128, 128), BF16, kind="ExternalInput").ap()
    out = nc.dram_tensor("out", (520, T), F32, kind="ExternalOutput").ap()
    with tile.TileContext(nc) as tc:
        _emit_body(tc, (xT, wqT, wkT, wvT, cosr, sins, p32, out), T)
    nc.compile()
    return nc


# ---------------- host side ----------------

def _qperm(j):
    rows = []
    for m in range(4):
        for r in range(128):
            h = m if r < 64 else m + 4
            d = 2 * (r % 32) + (1 if (r % 64) >= 32 else 0)
            rows.append((8 * j + h) * 64 + d)
    return np.array(rows)


def _kperm(j):
    rows = []
    for kv in range(2):
        for r in range(64):
            d = 2 * (r % 32) + (1 if r >= 32 else 0)
            rows.append((2 * j + kv) * 64 + d)
    return np.array(rows)


def make_core_inputs(x, Wq, Wk, Wv, cos, sin):
    """Per-core input dicts (host prep). x: [B,T,D]."""
    bf = ml_dtypes.bfloat16
    B, T, _ = x.shape
    xTb = [np.ascontiguousarray(x[b].T).astype(bf) for b in range(B)]
    cosT = np.ascontiguousarray(cos.T.astype(np.float32))
    sinT = np.ascontiguousarray(sin.T.astype(np.float32))
    cosr = np.tile(cosT, (4, 1)).astype(bf)
    sgn = np.repeat(np.array([-1.0, 1.0, -1.0, 1.0], np.float32), 32)
    sins = (np.tile(sinT, (4, 1)) * sgn[:, None]).astype(bf)
    p32 = np.zeros((128, 128), np.float32)
    swap = np.concatenate([np.arange(32, 64), np.arange(0, 32),
                           np.arange(96, 128), np.arange(64, 96)])
    p32[swap, np.arange(128)] = 1.0
    p32 = p32.astype(bf)
    maps = []
    for c in range(8):
        b, j = c // 4, c % 4
        maps.append({
            "xT": xTb[b],
            "wqT": np.ascontiguousarray(Wq[_qperm(j)].T).astype(bf),
            "wkT": np.ascontiguousarray(Wk[_kperm(j)].T).astype(bf),
            "wvT": np.ascontiguousarray(Wv[128 * j:128 * (j + 1)].T).astype(bf),
            "cosr": cosr,
            "sins": sins,
            "p32": p32,
        })
    return maps


def unshard(results, B=2, T=2048):
    """Combine per-core O^T blocks into the full [B, T, 2048] output."""
    out = np.empty((B, T, 2048), np.float32)
    for c in range(8):
        b, j = c // 4, c % 4
        blk = results[c]["out"].reshape(8, 65, T)
        O = blk[:, :64, :] / blk[:, 64:65, :]
        out[b, :, 512 * j:512 * (j + 1)] = (
            O.transpose(2, 0, 1).reshape(T, 512))
    return out


_CACHE = {}


def _get_program():
    if "nc" not in _CACHE:
        _CACHE["nc"] = build_program(T=2048, num_devices=8)
    return _CACHE["nc"]


def run_on_hw(in_maps, trace=False, tmpdir=None):
    nc = _get_program()
    return run_bass_kernel_spmd(nc, in_maps, list(range(8)), trace=trace,
                                tmpdir=tmpdir)


def kernel(x, Wq, Wk, Wv, cos, sin):
    x = np.asarray(x, np.float32)
    Wq = np.asarray(Wq, np.float32)
    Wk = np.asarray(Wk, np.float32)
    Wv = np.asarray(Wv, np.float32)
    cos = np.asarray(cos, np.float32)
    sin = np.asarray(sin, np.float32)
    maps = make_core_inputs(x, Wq, Wk, Wv, cos, sin)
    res = run_on_hw(maps, trace=False)
    return unshard(res.results, x.shape[0], x.shape[1])


# revision 35
# speedup vs baseline: 1.2104x; 1.0027x over previous
"""GQA kernel for Trainium2 (Bass/Tile), 8 NeuronCores — v4.

Sharding: core c -> batch b=c//4, kv-head pair j=c%4 (kv heads 2j,2j+1,
q heads 8j..8j+7).  Each core computes out[b, :, 512j:512(j+1)].

Structure (all compute bf16 / fp32-accum; fp8 was tested and rejected —
QK-fp8 alone costs 3.8e-2 rel err, P-fp8 5.5e-2, both over budget):
  * q-chunk-major attention: for j (512-wide q chunk) -> for pair
    (head m & m+4) -> for ki (k-tile): S^T chunk-pair, exp, PV.
  * S^T chunk-pair via row-tiled CONCURRENT matmuls: head m uses
    kt/qt partitions 0:64 (tile_position (0,0)), head m+4 uses 64:128
    ((64,0)) -> full PE array despite K=64 contraction.
  * causal mask: DVE adds -3e7 onto the diagonal 128x128 S^T block in
    PSUM before exp (exp then gives exact 0).
  * one exp ACTIVATE per item covers both heads ([128,2,w] strided AP
    over the 2-bank S pair); output bf16 into quad-plane pt tiles.
  * PV with V' stationary (V k-tile 64 cols + ones column, M=65),
    P^T chunk moving (N<=512) -> O^T[d|den, q] accumulates in PSUM
    over ki.  Stream-bound instead of LDWEIGHTS-bound.
  * V computed as V^T (wv stationary, x moving; stream-efficient),
    then PE-transposed per 128-col block into V' k-tiles.
  * output stays in O^T layout [8 heads x 65, T]; host divides by the
    denominator row and transposes.
  * projections split into 512-col chunk groups, woven through the
    attention stream as deadline-ordered fillers; input DMA ordered
    wk -> x.tc0 -> wq -> x rest -> wv across both HWDGE queues.
"""

import sys

for _p in ("/opt/trn_rl_repo",):
    if _p not in sys.path:
        sys.path.insert(0, _p)

import numpy as np
import ml_dtypes

import concourse.bass as bass
import concourse.tile as tile
from concourse import bacc, mybir
from concourse.bass_utils import run_bass_kernel_spmd
from concourse.masks import make_lower_triangular, make_identity

BF16 = mybir.dt.bfloat16
F32 = mybir.dt.float32
AF = mybir.ActivationFunctionType
ALU = mybir.AluOpType

D = 2048
HS = 64
SCALE = 0.125       # 1/sqrt(HS)
LAG = 3             # S->PV software-pipeline lag (items)


def _emit_body(tc, aps, T):
    nc = tc.nc
    NT = T // 128            # k tiles
    NJ = T // 512            # q chunks
    ND = D // 128            # contraction chunks

    xT, wqT, wkT, wvT, cosr, sins, p32, out = aps

    import contextlib
    ctx = tc._kernel_exitstack = contextlib.ExitStack()

    pers = ctx.enter_context(tc.tile_pool(name="pers", bufs=1))
    rp = ctx.enter_context(tc.tile_pool(name="rope", bufs=2))
    ptp = ctx.enter_context(tc.tile_pool(name="ptp", bufs=4))
    evp = ctx.enter_context(tc.tile_pool(name="evp", bufs=2))
    sp = ctx.enter_context(tc.tile_pool(name="spsum", bufs=2, space="PSUM"))
    po = ctx.enter_context(tc.tile_pool(name="opsum", bufs=1, space="PSUM"))
    aux = ctx.enter_context(tc.tile_pool(name="aux", bufs=1, space="PSUM"))

    # ---- input DMA: both HWDGE queues; priority order so K/Q0 start early
    wqTs, wkTs, wvTs = [], [], []
    xTs = [[None] * NJ for _ in range(ND)]
    for di in range(ND):
        t = pers.tile([128, 128], BF16, tag=f"wk{di}", name=f"wk{di}")
        nc.sync.dma_start(out=t[:], in_=wkT[di * 128:(di + 1) * 128, :])
        wkTs.append(t)
    for tcI in range(NJ):
        for di in range(ND):
            xTs[di][tcI] = pers.tile([128, 512], BF16, tag=f"x{di}_{tcI}",
                                     name=f"x{di}_{tcI}")

    def dma_x(q, di, tcI):
        eng = nc.sync if q == 0 else nc.gpsimd
        eng.dma_start(out=xTs[di][tcI][:],
                      in_=xT[di * 128:(di + 1) * 128,
                             tcI * 512:(tcI + 1) * 512])

    for di in range(ND):
        wqTs.append(pers.tile([128, 512], BF16, tag=f"wq{di}",
                              name=f"wq{di}"))
        wvTs.append(pers.tile([128, 128], BF16, tag=f"wv{di}",
                              name=f"wv{di}"))
    # sync (HWDGE): cos/sin chunk0 -> x.tc0 -> wq m0 -> wv -> xe.tc1 ->
    #               wq m1-3 -> xe.tc2-3.
    # gpsimd (SWDGE, slower, late-needed only): cos/sin rest, xo.tc1-3.
    cosr_t = pers.tile([128, T], BF16, tag="cosr", name="cosr")
    sins_t = pers.tile([128, T], BF16, tag="sins", name="sins")
    p32t = pers.tile([128, 128], BF16, tag="p32", name="p32")
    nc.sync.dma_start(out=p32t[:], in_=p32[:, :])
    nc.sync.dma_start(out=cosr_t[:, 0:512], in_=cosr[:, 0:512])
    nc.sync.dma_start(out=sins_t[:, 0:512], in_=sins[:, 0:512])
    for di in range(0, ND, 2):
        dma_x(0, di, 0)
    for di in range(1, ND, 2):
        dma_x(1, di, 0)
    # cos/sin tail after x.tc0 on the slow SWDGE queue: first needed by
    # rope of chunk 1 (~item 16), x.tc0 gates the very first projection.
    nc.gpsimd.dma_start(out=cosr_t[:, 512:T], in_=cosr[:, 512:T])
    nc.gpsimd.dma_start(out=sins_t[:, 512:T], in_=sins[:, 512:T])
    for di in range(ND):
        nc.sync.dma_start(out=wqTs[di][:, 0:128],
                          in_=wqT[di * 128:(di + 1) * 128, 0:128])
    for di in range(ND):
        nc.sync.dma_start(out=wvTs[di][:],
                          in_=wvT[di * 128:(di + 1) * 128, :])
    for di in range(0, ND, 2):
        dma_x(0, di, 1)
    for di in range(1, ND, 2):
        dma_x(1, di, 1)
    for di in range(ND):
        nc.sync.dma_start(out=wqTs[di][:, 128:512],
                          in_=wqT[di * 128:(di + 1) * 128, 128:512])
    for tcI in range(2, NJ):
        for di in range(0, ND, 2):
            dma_x(0, di, tcI)
        for di in range(1, ND, 2):
            dma_x(1, di, tcI)

    madd = pers.tile([128, 128], F32, tag="madd", name="madd")
    make_lower_triangular(nc, madd[:], val=-3.0e7, diag=False)
    idty = pers.tile([128, 128], BF16, tag="idty", name="idty")
    make_identity(nc, idty[:])

    qts = [pers.tile([128, T], BF16, tag=f"qt{m}", name=f"qt{m}")
           for m in range(4)]
    kt = pers.tile([128, T], BF16, tag="kt", name="kt")
    vxT = pers.tile([128, T], BF16, tag="vxT", name="vxT")
    vts = []
    for ti in range(NT):
        v = pers.tile([128, 130], BF16, tag=f"v{ti}", name=f"v{ti}")
        nc.vector.memset(v[:, 64:65], 1.0)
        nc.vector.memset(v[:, 129:130], 1.0)
        vts.append(v)

    # ---- helpers ----
    def rope_chunk(tgt, tcI, mirror=None):
        """RoPE on one 512-col chunk.  The 32-row-block swap is a PE
        permutation matmul (no DMA -> no queueing behind input loads)."""
        del mirror
        sl = slice(tcI * 512, (tcI + 1) * 512)
        ps = aux.tile([128, 512], F32, tag="pj", name="rsw")
        nc.tensor.matmul(ps[:], p32t[:], tgt[:, sl], start=True, stop=True)
        tmp = rp.tile([128, 512], BF16, tag="tmp", name="tmp")
        nc.vector.tensor_tensor(out=tmp[:], in0=tgt[:, sl], in1=cosr_t[:, sl],
                                op=ALU.mult)
        swp = rp.tile([128, 512], BF16, tag="swp", name="swp")
        nc.vector.tensor_tensor(out=swp[:], in0=ps[:], in1=sins_t[:, sl],
                                op=ALU.mult)
        nc.vector.tensor_tensor(out=tgt[:, sl], in0=tmp[:], in1=swp[:],
                                op=ALU.add)

    def make_quanta(kind, m, tcI):
        """One projection group (16 di-MMs into one [128,512] psum +
        evac), split into 4 quanta of 4 MMs."""
        state = {}

        def quantum(k):
            if k == 0:
                state["ps"] = aux.tile([128, 512], F32, tag="pj", name="pj")
            ps = state["ps"]
            for di in range(4 * k, 4 * k + 4):
                if kind == "q":
                    w = wqTs[di][:, m * 128:(m + 1) * 128]
                elif kind == "k":
                    w = wkTs[di][:]
                else:
                    w = wvTs[di][:]
                nc.tensor.matmul(ps[:], w, xTs[di][tcI][:],
                                 start=(di == 0), stop=(di == ND - 1))
            if k == 3:
                dst = {"q": qts[m] if m is not None else None,
                       "k": kt, "v": vxT}[kind]
                nc.vector.tensor_copy(dst[:, tcI * 512:(tcI + 1) * 512], ps[:])

        return [lambda kk=k: quantum(kk) for k in range(4)]

    def v_tr(ti):
        """PE-transpose one 128-col block of V^T into V' k-tile ti."""
        tp = aux.tile([128, 512], BF16, tag="vtr", name="vtr")
        nc.tensor.transpose(tp[:, 0:128], vxT[:, ti * 128:(ti + 1) * 128],
                            idty[:])
        nc.vector.tensor_copy(vts[ti][:, 0:64], tp[:, 0:64])
        nc.vector.tensor_copy(vts[ti][:, 65:129], tp[:, 64:128])

    # ---- prologue ----
    for f in make_quanta("k", None, 0):
        f()
    rope_chunk(kt, 0)
    for f in make_quanta("q", 0, 0):
        f()
    rope_chunk(qts[0], 0)
    for f in make_quanta("v", None, 0):
        f()
    for ti in range(4):
        v_tr(ti)

    # ---- filler queue ----
    def g_of(j, p):
        return sum(4 * jj + 4 for jj in range(j)) * 4 + (4 * j + 4) * p

    fillers = []

    def add_group(d, kind, m, tcI, rope_tgt=None, mirror=None):
        for f in make_quanta(kind, m, tcI):
            fillers.append((d, f))
        if rope_tgt is not None:
            fillers.append((d, lambda t=rope_tgt, c=tcI, mi=mirror:
                            rope_chunk(t, c, mi)))

    # arrival estimates (in items) for x chunk tcI: emitting a quantum
    # before its x DMA lands head-of-line-blocks the PE FIFO.
    arr = {0: 0, 1: 14, 2: 26, 3: 38}
    for tcI in range(1, NJ):
        d0 = g_of(tcI, 0)
        add_group(max(d0 - 8, arr[tcI]), "k", None, tcI, kt)
        add_group(max(d0 - 4, arr[tcI] + 1), "q", 0, tcI, qts[0])
        add_group(max(d0 + 2, arr[tcI] + 2), "v", None, tcI)
        for ti in range(4 * tcI, 4 * tcI + 4):
            fillers.append((max(d0 + 3, arr[tcI] + 3), lambda t=ti: v_tr(t)))
    for m in range(1, 4):
        for tcI in range(NJ):
            add_group(max(g_of(tcI, m) - 2, arr[tcI]), "q", m, tcI, qts[m])
    fillers.sort(key=lambda df: df[0])
    fq = list(fillers)

    # ---- attention stream ----
    items = [(j, p, ki) for j in range(NJ) for p in range(4)
             for ki in range(4 * j + 4)]
    pend = []
    acc = {}
    cur_pt = [None]

    def emit_S(j, p, ki):
        o = max(0, 128 * ki - 512 * j)
        s = sp.tile([128, 1024], F32, tag="s", name="s")
        q0 = 512 * j + o
        q1 = 512 * (j + 1)
        kc = slice(ki * 128, (ki + 1) * 128)
        nc.tensor.matmul(s[:, o:512], kt[0:64, kc],
                         qts[p][0:64, q0:q1], start=True, stop=True)
        nc.tensor.matmul(s[:, 512 + o:1024], kt[64:128, kc],
                         qts[p][64:128, q0:q1], start=True, stop=True)
        if ki >= 4 * j:  # diagonal k-tile: add -3e7 on the q<k triangle
            nc.vector.tensor_tensor(out=s[:, o:o + 128], in0=s[:, o:o + 128],
                                    in1=madd[:], op=ALU.add)
            nc.vector.tensor_tensor(out=s[:, 512 + o:512 + o + 128],
                                    in0=s[:, 512 + o:512 + o + 128],
                                    in1=madd[:], op=ALU.add)
        # pt pair tile: [128, 2048] bf16 = [A_e|A_o|B_e|B_o] planes
        if ki % 2 == 0:
            cur_pt[0] = ptp.tile([128, 2048], BF16, tag="pt", name="pt")
        pt = cur_pt[0]
        src = s[:].rearrange("p (g c) -> p g c", g=2)[:, :, o:512]
        half = pt[:].rearrange("p (g c) -> p g c", g=2)  # halves A / B
        if ki % 2 == 0:
            dst = half[:, :, o:512]
        else:
            dst = half[:, :, 512 + o:1024]
        nc.scalar.activation(dst, src, AF.Exp, scale=SCALE)
        return pt

    def emit_PV(j, p, ki, pt):
        o = max(0, 128 * ki - 512 * j)
        if ki == 0:
            acc[(j, p)] = (po.tile([65, 512], F32, tag="oA", name="oA"),
                           po.tile([65, 512], F32, tag="oB", name="oB"))
        oA, oB = acc[(j, p)]
        last = (ki == 4 * j + 3)
        quads = pt[:].rearrange("p (g c) -> p g c", g=4)
        qa = ki % 2          # plane within pt for this ki
        nc.tensor.matmul(oA[:, o:512], vts[ki][:, 0:65],
                         quads[:, qa, o:512],
                         start=(ki == 0), stop=last)
        nc.tensor.matmul(oB[:, o:512], vts[ki][:, 65:130],
                         quads[:, 2 + qa, o:512],
                         start=(ki == 0), stop=last)
        if last:
            del acc[(j, p)]
            otA = evp.tile([65, 512], F32, tag="oa", name="oa")
            nc.vector.tensor_copy(otA[:], oA[:])
            nc.sync.dma_start(
                out=out[65 * p:65 * (p + 1), 512 * j:512 * (j + 1)],
                in_=otA[:])
            otB = evp.tile([65, 512], F32, tag="ob", name="ob")
            nc.vector.tensor_copy(otB[:], oB[:])
            nc.sync.dma_start(
                out=out[65 * (p + 4):65 * (p + 5), 512 * j:512 * (j + 1)],
                in_=otB[:])

    for g, (j, p, ki) in enumerate(items):
        pt = emit_S(j, p, ki)
        pend.append((j, p, ki, pt))
        if len(pend) > LAG:
            emit_PV(*pend.pop(0))
        # emit fillers: force anything due, spread the rest one per item
        # but never look further ahead than ~6 items (premature emission
        # of a quantum whose x chunk hasn't landed stalls the PE FIFO).
        emitted = 0
        while fq and (fq[0][0] <= g + 2 or
                      (emitted < 1 and fq[0][0] <= g + 6)):
            fq.pop(0)[1]()
            emitted += 1
    while pend:
        emit_PV(*pend.pop(0))
    while fq:
        fq.pop(0)[1]()

    ctx.close()


def build_program(T=2048, num_devices=8):
    nc = bacc.Bacc("TRN2", target_bir_lowering=False, debug=False,
                   num_devices=num_devices)
    xT = nc.dram_tensor("xT", (D, T), BF16, kind="ExternalInput").ap()
    wqT = nc.dram_tensor("wqT", (D, 512), BF16, kind="ExternalInput").ap()
    wkT = nc.dram_tensor("wkT", (D, 128), BF16, kind="ExternalInput").ap()
    wvT = nc.dram_tensor("wvT", (D, 128), BF16, kind="ExternalInput").ap()
    cosr = nc.dram_tensor("cosr", (128, T), BF16, kind="ExternalInput").ap()
    sins = nc.dram_tensor("sins", (128, T), BF16, kind="ExternalInput").ap()
    p32 = nc.dram_tensor("p32", (128, 128), BF16, kind="ExternalInput").ap()
    out = nc.dram_tensor("out", (520, T), F32, kind="ExternalOutput").ap()
    with tile.TileContext(nc) as tc:
        _emit_body(tc, (xT, wqT, wkT, wvT, cosr, sins, p32, out), T)
    nc.compile()
    return nc


# ---------------- host side ----------------

def _qperm(j):
    rows = []
    for m in range(4):
        for r in range(128):
            h = m if r < 64 else m + 4
            d = 2 * (r % 32) + (1 if (r % 64) >= 32 else 0)
            rows.append((8 * j + h) * 64 + d)
    return np.array(rows)


def _kperm(j):
    rows = []
    for kv in range(2):
        for r in range(64):
            d = 2 * (r % 32) + (1 if r >= 32 else 0)
            rows.append((2 * j + kv) * 64 + d)
    return np.array(rows)


def make_core_inputs(x, Wq, Wk, Wv, cos, sin):
    """Per-core input dicts (host prep). x: [B,T,D]."""
    bf = ml_dtypes.bfloat16
    B, T, _ = x.shape
    xTb = [np.ascontiguousarray(x[b].T).astype(bf) for b in range(B)]
    cosT = np.ascontiguousarray(cos.T.astype(np.float32))
    sinT = np.ascontiguousarray(sin.T.astype(np.float32))
    cosr = np.tile(cosT, (4, 1)).astype(bf)
    sgn = np.repeat(np.array([-1.0, 1.0, -1.0, 1.0], np.float32), 32)
    sins = (np.tile(sinT, (4, 1)) * sgn[:, None]).astype(bf)
    p32 = np.zeros((128, 128), np.float32)
    swap = np.concatenate([np.arange(32, 64), np.arange(0, 32),
                           np.arange(96, 128), np.arange(64, 96)])
    p32[swap, np.arange(128)] = 1.0
    p32 = p32.astype(bf)
    maps = []
    for c in range(8):
        b, j = c // 4, c % 4
        maps.append({
            "xT": xTb[b],
            "wqT": np.ascontiguousarray(Wq[_qperm(j)].T).astype(bf),
            "wkT": np.ascontiguousarray(Wk[_kperm(j)].T).astype(bf),
            "wvT": np.ascontiguousarray(Wv[128 * j:128 * (j + 1)].T).astype(bf),
            "cosr": cosr,
            "sins": sins,
            "p32": p32,
        })
    return maps


def unshard(results, B=2, T=2048):
    """Combine per-core O^T blocks into the full [B, T, 2048] output."""
    out = np.empty((B, T, 2048), np.float32)
    for c in range(8):
        b, j = c // 4, c % 4
        blk = results[c]["out"].reshape(8, 65, T)
        O = blk[:, :64, :] / blk[:, 64:65, :]
        out[b, :, 512 * j:512 * (j + 1)] = (
            O.transpose(2, 0, 1).reshape(T, 512))
    return out


_CACHE = {}


def _get_program():
    if "nc" not in _CACHE:
        _CACHE["nc"] = build_program(T=2048, num_devices=8)
    return _CACHE["nc"]


def run_on_hw(in_maps, trace=False, tmpdir=None):
    nc = _get_program()
    return run_bass_kernel_spmd(nc, in_maps, list(range(8)), trace=trace,
                                tmpdir=tmpdir)


def kernel(x, Wq, Wk, Wv, cos, sin):
    x = np.asarray(x, np.float32)
    Wq = np.asarray(Wq, np.float32)
    Wk = np.asarray(Wk, np.float32)
    Wv = np.asarray(Wv, np.float32)
    cos = np.asarray(cos, np.float32)
    sin = np.asarray(sin, np.float32)
    maps = make_core_inputs(x, Wq, Wk, Wv, cos, sin)
    res = run_on_hw(maps, trace=False)
    return unshard(res.results, x.shape[0], x.shape[1])
